# revision 2
# baseline (speedup 1.0000x reference)
"""CRF loss (sum of log-likelihoods) on 8 Trainium2 NeuronCores.

Shapes (hardcoded): emissions (512, 512, 128) f32, tags (512, 512) i64,
mask (512, 512) bool (all ones), start/end (128,) f32, transitions
(128, 128) f32.  Output: scalar f32 = sum_b llh_b.

Device computes ONLY the partition function (forward DP); the path score
(numerator) is a pure gather+sum done host-side in float64.

DP strategy: data-parallel over batch (64 seqs/core) AND chunk-parallel in
time.  E = exp(trans), |trans| <= 0.1, is a strong Hilbert contraction
(factor ~0.1/step), so the normalized forward vector forgets its init in a
few steps.  T=512 is split into C=32 chunks of L=16; each chunk warms up W
steps from a uniform vector using the x of the W timesteps preceding it
(read at a -64 column shift), then chunk contributions telescope:
  logZ_b = sum_c [ln(1^T u_c) - ln(1^T y_c)] + T*G
where y_c is the post-warmup vector, u_c the chunk result, and G a constant
per-step normalizer folded into x = exp(em - G).  start/end are folded into
the first/last x slabs; chunk 0 is initialized exactly from the t=0 slab
(its ln(1^T y) term is 0).  Raw sums 1^T y and 1^T u ship back as f32 rows;
the host takes logs in f64.

Per wide step (2048 columns = 32 chunks x 64 seqs) the work is split into
5 independent chains sized so the three multiply pipelines finish together:
3 "v" chains (PE matmul -> DVE multiply from PSUM) and 2 "g" chains (PE ->
ACT copy to SBUF -> GPSIMD multiply).  Chain columns: 3x427 + 383 + 384.
"""

import numpy as np

B, T, K = 512, 512, 128
NCORES = 8
BC = B // NCORES          # 64 sequences per core
C = 32                    # time chunks
L = T // C                # 16 steps per chunk
W = 3                     # warmup steps per chunk
NSTEP = L + W
CBC = C * BC              # 2048 columns per wide step
SLAB = CBC
G = 4.85                  # per-step growth normalizer

# [start, end, engine]: "v" = DVE multiplies S (PSUM) by x directly;
# "g" = ACT copies S to SBUF bf16, Pool (gpsimd) multiplies.
CHAINS = [(0, 427, "v"), (427, 854, "v"), (854, 1281, "v"),
          (1281, 1664, "g"), (1664, 2048, "g")]

_PROGRAM = None


def _build_program(nstep=NSTEP, chains=CHAINS, with_num=True):
    from contextlib import ExitStack

    import concourse.bacc as bacc
    import concourse.mybir as mybir
    import concourse.tile as tile

    f32 = mybir.dt.float32
    bf16 = mybir.dt.bfloat16
    fp8e5 = mybir.dt.float8e5

    nc = bacc.Bacc("TRN2", target_bir_lowering=False)

    xS_d = nc.dram_tensor("xS", [L, K, CBC], fp8e5, kind="ExternalInput")
    Eb_d = nc.dram_tensor("Eb", [K, K], bf16, kind="ExternalInput")

    r0_d = nc.dram_tensor("r0r", [1, CBC], f32, kind="ExternalOutput")
    rF_d = nc.dram_tensor("rFr", [1, CBC], f32, kind="ExternalOutput")

    XSPLIT = 1281             # first-slab DMA split (both halves >= 512B)

    with tile.TileContext(nc) as tc, ExitStack() as ctx:
        const = ctx.enter_context(tc.tile_pool(name="const", bufs=1))
        big = ctx.enter_context(tc.tile_pool(name="big", bufs=1))
        p_pool = ctx.enter_context(tc.tile_pool(name="pp", bufs=2))
        sb_pool = ctx.enter_context(tc.tile_pool(name="sbp", bufs=2))
        small = ctx.enter_context(tc.tile_pool(name="small", bufs=1))
        spsum = ctx.enter_context(tc.tile_pool(name="spsum", bufs=1, space="PSUM"))
        stage = ctx.enter_context(tc.tile_pool(name="stage", bufs=2, space="PSUM"))

        # ---------------- constants ----------------
        E_sb = const.tile([K, K], bf16, tag="E")
        nc.sync.dma_start(E_sb[:], Eb_d[:])
        ones_col = const.tile([K, 1], bf16, tag="ones_col")
        nc.vector.memset(ones_col[:], 1.0)

        # result rows (memset so unwritten cols stay finite)
        r0row = small.tile([1, CBC], f32, tag="r0row")
        rFrow = small.tile([1, CBC], f32, tag="rFrow")
        nc.vector.memset(r0row[:], 1.0)
        nc.vector.memset(rFrow[:], 1.0)

        # ---------------- x stream ----------------
        x_sb = big.tile([K, L * SLAB], fp8e5, tag="x")

        def dma_slab_split(p):
            for c0, c1 in ((0, XSPLIT), (XSPLIT, CBC)):
                nc.sync.dma_start(
                    x_sb[:, p * SLAB + c0 : p * SLAB + c1],
                    xS_d[p, :, c0:c1],
                )

        def dma_block(a, b):
            nc.sync.dma_start(
                x_sb[:, a * SLAB : b * SLAB].rearrange(
                    "k (p cb) -> k p cb", p=b - a),
                xS_d[a:b].rearrange("p k cb -> k p cb"),
            )

        dma_slab_split(0)
        dma_slab_split(1)
        for j in range(2, L, 2):
            dma_block(j, j + 2)

        # ---------------- state init ----------------
        P = {}
        for gi, (g0, g1, eng) in enumerate(chains):
            P[gi] = p_pool.tile([K, g1 - g0], bf16, tag=f"P{gi}", name=f"Pinit{gi}")
            nc.vector.memset(P[gi][:], 1.0)

        def emit_r(gi, dst_row, is_final):
            g0, g1, eng = chains[gi]
            lo = 64 if (g0 == 0 and not is_final) else 0
            rb = stage.tile([1, 512], f32, tag="rstage", name=f"r{int(is_final)}_{gi}")
            nc.tensor.matmul(rb[:, : g1 - g0 - lo], lhsT=ones_col[:],
                             rhs=P[gi][:, lo:], start=True, stop=True)
            # evict PSUM row -> SBUF row slice (ACT for g, DVE for v chains
            # at the tail; mid-loop all on ACT to keep DVE on the DP)
            if is_final and eng == "v":
                nc.vector.tensor_copy(dst_row[0:1, g0 + lo : g1],
                                      rb[:, : g1 - g0 - lo])
            else:
                nc.scalar.copy(dst_row[0:1, g0 + lo : g1], rb[:, : g1 - g0 - lo])

        # ---------------- main DP ----------------
        order = sorted(range(len(chains)), key=lambda gi: chains[gi][2] != "g")
        for s in range(nstep):
            q = s if s <= L - 1 else s - L
            shift = 64 if s < W else 0
            for gi in order:
                g0, g1, eng = chains[gi]
                lo = 64 if (s <= W and g0 == 0) else 0
                S = spsum.tile([K, g1 - g0], f32, tag=f"S{gi}", name=f"S{gi}_{s}")
                nc.tensor.matmul(S[:, lo:], lhsT=E_sb[:], rhs=P[gi][:, lo:],
                                 start=True, stop=True)
                Pn = p_pool.tile([K, g1 - g0], bf16, tag=f"P{gi}", name=f"P{gi}_{s}")
                xa = x_sb[:, q * SLAB + g0 + lo - shift : q * SLAB + g1 - shift]
                if eng == "v":
                    nc.vector.tensor_mul(Pn[:, lo:], S[:, lo:], xa)
                else:
                    Sb = sb_pool.tile([K, g1 - g0], bf16, tag=f"Sb{gi}",
                                      name=f"Sb{gi}_{s}")
                    nc.scalar.copy(Sb[:, lo:], S[:, lo:])
                    nc.gpsimd.tensor_mul(Pn[:, lo:], Sb[:, lo:], xa)
                if s == W and g0 == 0:
                    # chunk-0 exact init from the t=0 slab (start folded in)
                    nc.vector.tensor_copy(Pn[:, 0:64],
                                          x_sb[:, W * SLAB : W * SLAB + 64])
                P[gi] = Pn
                if s == W - 1:
                    emit_r(gi, r0row, is_final=False)

        nc.sync.dma_start(r0_d[:], r0row[:])

        for gi in range(len(chains)):
            emit_r(gi, rFrow, is_final=True)
        nc.sync.dma_start(rF_d[:], rFrow[:])

    nc.compile()
    return nc


def _host_score(em, tg, start, end, trans):
    """Exact path score, float64, vectorized."""
    em = em.astype(np.float64)
    barange = np.arange(em.shape[0])
    score = start.astype(np.float64)[tg[:, 0]] + em[barange, 0, tg[:, 0]]
    emit = np.take_along_axis(em, tg[..., None], axis=2)[..., 0]     # (B, T)
    trans_sc = trans.astype(np.float64)[tg[:, :-1], tg[:, 1:]]       # (B, T-1)
    score = score + (trans_sc + emit[:, 1:]).sum(axis=1)
    score = score + end.astype(np.float64)[tg[:, -1]]
    return score                                                     # (B,)


def _prep_inputs(emissions, start_transitions, end_transitions, transitions):
    import concourse.mybir as mybir

    bf16 = mybir.dt.np(mybir.dt.bfloat16)
    fp8e5 = mybir.dt.np(mybir.dt.float8e5)

    em = np.asarray(emissions, dtype=np.float32)         # (B, T, K)
    start = np.asarray(start_transitions, dtype=np.float32)
    end = np.asarray(end_transitions, dtype=np.float32)
    trans = np.asarray(transitions, dtype=np.float32)

    emx = em.copy()
    emx[:, 0, :] += start[None, :]
    emx[:, -1, :] += end[None, :]
    x_full = np.exp(emx - G)

    # step-major slabs in position order perm = [L-W..L-1, 0..L-W-1]
    # (warmup steps reuse the previous chunk's tail slabs at a -64 shift)
    perm = np.array(list(range(L - W, L)) + list(range(L - W)))

    common = {"Eb": np.exp(trans).astype(bf16)}
    in_maps = []
    for cr in range(NCORES):
        a = x_full[cr * BC : (cr + 1) * BC]              # (BC, T, K)
        s = a.reshape(BC, C, L, K).transpose(2, 3, 1, 0).reshape(L, K, CBC)
        m = dict(common)
        m["xS"] = np.ascontiguousarray(s[perm]).astype(fp8e5)
        in_maps.append(m)
    return in_maps


def kernel(emissions, tags, mask, start_transitions, end_transitions, transitions,
           trace=False):
    global _PROGRAM
    from concourse.bass_utils import run_bass_kernel_spmd

    mask_np = np.asarray(mask)
    assert mask_np.all(), "kernel assumes an all-ones mask"

    em = np.asarray(emissions, dtype=np.float32)
    tg = np.asarray(tags).astype(np.int64)
    start = np.asarray(start_transitions, dtype=np.float32)
    end = np.asarray(end_transitions, dtype=np.float32)
    trans = np.asarray(transitions, dtype=np.float32)

    score = _host_score(em, tg, start, end, trans)       # (B,) f64

    in_maps = _prep_inputs(em, start, end, trans)
    if _PROGRAM is None:
        _PROGRAM = _build_program()

    res = run_bass_kernel_spmd(
        _PROGRAM, in_maps, core_ids=list(range(NCORES)), trace=trace
    )
    total = np.float64(score.sum(dtype=np.float64))
    for r in res.results:
        r0 = r["r0r"].astype(np.float64).reshape(CBC)
        rF = r["rFr"].astype(np.float64).reshape(CBC)
        r0[:BC] = 1.0                                    # chunk 0: exact init
        # logZ summed over this core's 64 seqs and all 32 chunks
        total -= np.log(rF).sum() - np.log(r0).sum() + BC * T * G
    kernel.last_results = res
    return np.float32(total)


# revision 7
# speedup vs baseline: 1.3662x; 1.3662x over previous
"""CRF loss (sum of log-likelihoods) on 8 Trainium2 NeuronCores.

Shapes (hardcoded): emissions (512, 512, 128) f32, tags (512, 512) i64,
mask (512, 512) bool (all ones), start/end (128,) f32, transitions
(128, 128) f32.  Output: scalar f32 = sum_b llh_b.

Device computes ONLY the partition function (forward DP); the path score
(numerator) is a pure gather+sum done host-side in float64.

DP strategy: data-parallel over batch (64 seqs/core) AND chunk-parallel in
time.  E = exp(trans), |trans| <= 0.1, is a strong Hilbert contraction
(factor ~0.1/step), so the forward vector forgets its initial condition in
a few steps.  T=512 is split into C=64 chunks of L=8; every chunk (except
chunk 0, which is initialized exactly from the t=0 slab) starts from the
uniform vector with NO warmup — the restart bias is far below the fp8
noise floor (validated: rel err 5.4e-4, identical to a 3-step warmup).
Chunk contributions telescope:
  logZ_b = sum_c [ln(1^T u_c) - ln r0_c] + T*G
with r0_c = K = 128 for c >= 1 (uniform init) and 1 for c = 0, and G a
constant per-step normalizer folded into x = exp(em - G).  start/end are
folded into the first/last x slabs.  Raw sums 1^T u ship back as an f32
row; the host takes logs in f64.

Per wide step (4096 columns = 64 chunks x 64 seqs) the work is split into
4 independent chains sized to fill PSUM's 8 banks exactly and to balance
engine occupancy against the per-chain round-trip latency: 2 "v" chains
(PE matmul -> DVE multiply from PSUM, 1536 cols, 3 banks) and 2 "g" chains
(PE -> ACT copy to SBUF -> GPSIMD multiply, 512 cols, 1 bank).
"""

import numpy as np

B, T, K = 512, 512, 128
NCORES = 8
BC = B // NCORES          # 64 sequences per core
C = 64                    # time chunks
L = T // C                # 8 steps per chunk
W = 0                     # no warmup needed (contraction >> restart bias)
NSTEP = L + W
CBC = C * BC              # 4096 columns per wide step
SLAB = CBC
G = 4.85                  # per-step growth normalizer

# [start, end, engine]: "v" = DVE multiplies S (PSUM) by x directly;
# "g" = ACT copies S to SBUF bf16, Pool (gpsimd) multiplies.
CHAINS = [(0, 1536, "v"), (1536, 3072, "v"),
          (3072, 3584, "g"), (3584, 4096, "g")]
SWIDTH = {"v": 1536, "g": 512}

_PROGRAM = None


def _build_program(nstep=NSTEP, chains=CHAINS, with_num=True):
    from contextlib import ExitStack

    import concourse.bacc as bacc
    import concourse.mybir as mybir
    import concourse.tile as tile

    f32 = mybir.dt.float32
    bf16 = mybir.dt.bfloat16
    fp8e5 = mybir.dt.float8e5

    nc = bacc.Bacc("TRN2", target_bir_lowering=False)

    xS_d = nc.dram_tensor("xS", [L, K, CBC], fp8e5, kind="ExternalInput")
    Eb_d = nc.dram_tensor("Eb", [K, K], bf16, kind="ExternalInput")

    rF_d = nc.dram_tensor("rFr", [1, CBC], f32, kind="ExternalOutput")

    with tile.TileContext(nc) as tc, ExitStack() as ctx:
        const = ctx.enter_context(tc.tile_pool(name="const", bufs=1))
        big = ctx.enter_context(tc.tile_pool(name="big", bufs=1))
        p_pool = ctx.enter_context(tc.tile_pool(name="pp", bufs=2))
        sb_pool = ctx.enter_context(tc.tile_pool(name="sbp", bufs=2))
        small = ctx.enter_context(tc.tile_pool(name="small", bufs=1))
        spsum = ctx.enter_context(tc.tile_pool(name="spsum", bufs=1, space="PSUM"))

        # ---------------- constants ----------------
        E_sb = const.tile([K, K], bf16, tag="E")
        nc.sync.dma_start(E_sb[:], Eb_d[:])
        ones_col = const.tile([K, 1], bf16, tag="ones_col")
        nc.vector.memset(ones_col[:], 1.0)

        rFrow = small.tile([1, CBC], f32, tag="rFrow")

        # ---------------- x stream ----------------
        x_sb = big.tile([K, L * SLAB], fp8e5, tag="x")

        def dma_slab_split(p):
            # split along chain boundaries so chains can start independently
            for c0, c1 in ((0, 1536), (1536, 3072), (3072, CBC)):
                nc.sync.dma_start(
                    x_sb[:, p * SLAB + c0 : p * SLAB + c1],
                    xS_d[p, :, c0:c1],
                )

        dma_slab_split(0)
        dma_slab_split(1)
        for j in range(2, L, 2):
            nc.sync.dma_start(
                x_sb[:, j * SLAB : (j + 2) * SLAB].rearrange(
                    "k (p cb) -> k p cb", p=2),
                xS_d[j : j + 2].rearrange("p k cb -> k p cb"),
            )

        # ---------------- state tiles ----------------
        S = {}
        P = {}
        for gi, (g0, g1, eng) in enumerate(chains):
            F = g1 - g0
            S[gi] = spsum.tile([K, SWIDTH[eng]], f32, tag=f"S{gi}", name=f"S{gi}")
            P[gi] = p_pool.tile([K, F], bf16, tag=f"P{gi}", name=f"Pinit{gi}")
            if eng == "v":
                nc.vector.memset(P[gi][:], 1.0)
            else:
                nc.gpsimd.memset(P[gi][:], 1.0)

        # ---------------- main DP ----------------
        def emit_step(gi, s):
            g0, g1, eng = chains[gi]
            F = g1 - g0
            lo = 64 if (s == 0 and g0 == 0) else 0
            for a in range(lo, F, 512):
                b = min(a + 512, F)
                a = max(a, lo)
                nc.tensor.matmul(S[gi][:, a:b], lhsT=E_sb[:],
                                 rhs=P[gi][:, a:b], start=True, stop=True)
            Pn = p_pool.tile([K, F], bf16, tag=f"P{gi}", name=f"P{gi}_{s}")
            xa = x_sb[:, s * SLAB + g0 + lo : s * SLAB + g1]
            if eng == "v":
                nc.vector.tensor_mul(Pn[:, lo:], S[gi][:, lo:F], xa)
            else:
                Sb = sb_pool.tile([K, F], bf16, tag=f"Sb{gi}",
                                  name=f"Sb{gi}_{s}")
                nc.scalar.copy(Sb[:, lo:], S[gi][:, lo:F])
                nc.gpsimd.tensor_mul(Pn[:, lo:], Sb[:, lo:], xa)
            if s == 0 and g0 == 0:
                # chunk-0 exact init from the t=0 slab (start folded in)
                nc.vector.tensor_copy(Pn[:, 0:64], x_sb[:, 0:64])
            P[gi] = Pn

        # g-chains one step behind the v-chains: matmuls stalled at the head
        # of the in-order PE queue always have ready instructions behind them
        v_chains = [gi for gi, ch in enumerate(chains) if ch[2] == "v"]
        g_chains = [gi for gi, ch in enumerate(chains) if ch[2] == "g"]
        for s in range(nstep + 1):
            if s < nstep:
                for gi in v_chains:
                    emit_step(gi, s)
            if s >= 1:
                for gi in g_chains:
                    emit_step(gi, s - 1)

        # ---------------- final sums ----------------
        # S banks are dead after the last mult: stage rF in-place.
        for gi, (g0, g1, eng) in enumerate(chains):
            F = g1 - g0
            for a in range(0, F, 512):
                b = min(a + 512, F)
                nc.tensor.matmul(S[gi][0:1, a:b], lhsT=ones_col[:],
                                 rhs=P[gi][:, a:b], start=True, stop=True)
        for gi, (g0, g1, eng) in enumerate(chains):
            F = g1 - g0
            if gi in (0, 2):
                nc.vector.tensor_copy(rFrow[0:1, g0:g1], S[gi][0:1, 0:F])
            else:
                nc.scalar.copy(rFrow[0:1, g0:g1], S[gi][0:1, 0:F])
        nc.sync.dma_start(rF_d[:], rFrow[:])

    nc.compile()
    return nc


def _host_score(em, tg, start, end, trans):
    """Exact path score, float64, vectorized."""
    em = em.astype(np.float64)
    barange = np.arange(em.shape[0])
    score = start.astype(np.float64)[tg[:, 0]] + em[barange, 0, tg[:, 0]]
    emit = np.take_along_axis(em, tg[..., None], axis=2)[..., 0]     # (B, T)
    trans_sc = trans.astype(np.float64)[tg[:, :-1], tg[:, 1:]]       # (B, T-1)
    score = score + (trans_sc + emit[:, 1:]).sum(axis=1)
    score = score + end.astype(np.float64)[tg[:, -1]]
    return score                                                     # (B,)


def _prep_inputs(emissions, start_transitions, end_transitions, transitions):
    import concourse.mybir as mybir

    bf16 = mybir.dt.np(mybir.dt.bfloat16)
    fp8e5 = mybir.dt.np(mybir.dt.float8e5)

    em = np.asarray(emissions, dtype=np.float32)         # (B, T, K)
    start = np.asarray(start_transitions, dtype=np.float32)
    end = np.asarray(end_transitions, dtype=np.float32)
    trans = np.asarray(transitions, dtype=np.float32)

    emx = em.copy()
    emx[:, 0, :] += start[None, :]
    emx[:, -1, :] += end[None, :]
    x_full = np.exp(emx - G)

    common = {"Eb": np.exp(trans).astype(bf16)}
    in_maps = []
    for cr in range(NCORES):
        a = x_full[cr * BC : (cr + 1) * BC]              # (BC, T, K)
        s = a.reshape(BC, C, L, K).transpose(2, 3, 1, 0).reshape(L, K, CBC)
        m = dict(common)
        m["xS"] = np.ascontiguousarray(s).astype(fp8e5)
        in_maps.append(m)
    return in_maps


def kernel(emissions, tags, mask, start_transitions, end_transitions, transitions,
           trace=False):
    global _PROGRAM
    from concourse.bass_utils import run_bass_kernel_spmd

    mask_np = np.asarray(mask)
    assert mask_np.all(), "kernel assumes an all-ones mask"

    em = np.asarray(emissions, dtype=np.float32)
    tg = np.asarray(tags).astype(np.int64)
    start = np.asarray(start_transitions, dtype=np.float32)
    end = np.asarray(end_transitions, dtype=np.float32)
    trans = np.asarray(transitions, dtype=np.float32)

    score = _host_score(em, tg, start, end, trans)       # (B,) f64

    in_maps = _prep_inputs(em, start, end, trans)
    if _PROGRAM is None:
        _PROGRAM = _build_program()

    res = run_bass_kernel_spmd(
        _PROGRAM, in_maps, core_ids=list(range(NCORES)), trace=trace
    )
    total = np.float64(score.sum(dtype=np.float64))
    # per sequence: logZ = sum_c ln(rF_c) - (C-1)*ln(K) + T*G
    lnr0 = (C - 1) * np.log(np.float64(K))
    for r in res.results:
        rF = r["rFr"].astype(np.float64).reshape(CBC)
        total -= np.log(rF).sum() + BC * (T * G - lnr0)
    kernel.last_results = res
    return np.float32(total)
